# revision 9
# baseline (speedup 1.0000x reference)
"""MemoryBank kernel for 8x Trainium2 NeuronCores.

reference:
    similarity = q [B,T,D] @ mem^T [D,M]          (B,T,D,M = 16,2048,128,4096)
    attention  = softmax(similarity, axis=-1)      -> output 2 [B,T,M] f32
    mem_feats  = attention @ mem                   -> output 1 [B,T,D] f32

Sharding: pure data-parallel over batch. Each of the 8 cores handles 2
batches (4096 rows), with the 4096x128 memory bank replicated. No
collectives; host concatenates the per-core outputs.

Per-core algorithm (rows tiled by 128):
    mm1  (PE, fp16):  sim[rows=128p, m] = qT.T @ memT   (PSUM f32)
    exp  (ACT):       e = exp(sim) -> SBUF fp16, fused row-sum (accum_out)
                      (no max subtraction: logits are bounded ~|4|)
    norm (DVE):       attn_f32 = e * (1/sum)  -> DMA out
    trans(PE):        e -> e^T tiles (fp16, batched through PSUM)
    mm2  (PE, fp16):  mfT[d=128p, rows] += mem[m,d].T-contraction @ e^T
    tail (PE/DVE):    mfT -> transpose -> * (1/sum) -> DMA out
"""

import numpy as np

B, T, D, M = 16, 2048, 128, 4096
N_CORES = 8
P = 128
R = (B // N_CORES) * T          # rows per core = 4096
RT = R // P                     # 32 row tiles
MO = M // P                     # 32 memory chunks
BLK = 4                         # row tiles per mm2 block
NB = RT // BLK                  # 8 blocks
TRG = 16                        # transposes per PSUM batch group

_CACHE = {}


def _split_multiwaits(nc, mybir, max_waits=2):
    """walrus in this container encodes a limited number of sync waits per
    instruction (1 for matmul, ~2 for most others); Tile can attach more.
    Split the excess into preceding NOPs on the same engine."""
    n = 0
    for fn in nc.m.functions:
        for b in fn.blocks:
            insts = b.instructions
            i = 0
            while i < len(insts):
                inst = insts[i]
                lim = (
                    1
                    if isinstance(
                        inst,
                        (
                            mybir.InstMatmult,
                            mybir.InstLdweights,
                            mybir.InstActivation,
                        ),
                    )
                    else max_waits
                )
                si = inst.sync_info
                if si is not None and si.on_wait and len(si.on_wait) > lim:
                    waits = list(si.on_wait)
                    keep = waits[-lim:]
                    extra = waits[:-lim]
                    inst.sync_info = mybir.SyncInfo(
                        on_wait=keep, on_update=si.on_update
                    )
                    for j, w in enumerate(extra):
                        nop = mybir.InstEventSemaphore(
                            name=f"{inst.name}_wsplit{j}",
                            engine=inst.engine,
                            ins=[],
                            outs=[],
                            sync_info=mybir.SyncInfo(on_wait=[w], on_update=[]),
                        )
                        insts.insert(i, nop)
                        i += 1
                        n += 1
                i += 1
    return n


def build_nc(norm_on_act_every=0, fixup=True):
    """Build the per-core Bass module.

    norm_on_act_every: if k>0, every k-th row tile's normalize runs on the
    scalar engine (ACT) instead of DVE, to balance engine load.
    """
    import concourse.bass as bass
    import concourse.mybir as mybir
    from concourse.tile import TileContext
    from concourse.masks import make_identity

    f32 = mybir.dt.float32
    f16 = mybir.dt.float16
    EXP = mybir.ActivationFunctionType.Exp
    COPY = mybir.ActivationFunctionType.Copy

    nc = bass.Bass()
    q = nc.dram_tensor("q", [R, D], f32, kind="ExternalInput")
    mem = nc.dram_tensor("mem", [M, D], f32, kind="ExternalInput")
    attn = nc.dram_tensor("attn", [R, M], f32, kind="ExternalOutput")
    mf = nc.dram_tensor("mf", [R, D], f32, kind="ExternalOutput")

    with TileContext(nc) as tc:
        with (
            tc.tile_pool(name="persist", bufs=1) as persist,
            tc.tile_pool(name="stats", bufs=4) as stats,
            tc.tile_pool(name="expp", bufs=3) as expp,
            tc.tile_pool(name="attnp", bufs=3) as attnp,
            tc.tile_pool(name="expTp", bufs=2) as expTp,
            tc.tile_pool(name="mfp", bufs=2) as mfp,
            tc.tile_pool(name="psum_sim", bufs=1, space="PSUM") as psum_sim,
            tc.tile_pool(name="psum_tr", bufs=1, space="PSUM") as psum_tr,
            tc.tile_pool(name="psum_mf", bufs=1, space="PSUM") as psum_mf,
            tc.tile_pool(name="psum_mft", bufs=1, space="PSUM") as psum_mft,
        ):
            # ---------------- prep ----------------
            ident16 = persist.tile([P, P], f16, tag="ident16")
            make_identity(nc, ident16)
            ident32 = persist.tile([P, P], f32, tag="ident32")
            make_identity(nc, ident32)

            # fp16 copies of mem and q, partition-tiled (cast during DMA)
            mem16 = persist.tile([P, MO, D], f16, tag="mem16")
            nc.gpsimd.dma_start(mem16[:], mem.rearrange("(mo p) d -> p mo d", p=P))
            q16 = persist.tile([P, RT, D], f16, tag="q16")
            nc.gpsimd.dma_start(q16[:], q.rearrange("(rt p) d -> p rt d", p=P))

            # memT [d=128, m=4096] and qT [d=128, rt, rows=128] via PE transpose
            memT = persist.tile([P, MO * P], f16, tag="memT")
            qT = persist.tile([P, RT, P], f16, tag="qT")
            for g in range(MO // TRG):
                tr = psum_tr.tile([P, TRG * P], f16, tag="tr")
                for k in range(TRG):
                    mo = g * TRG + k
                    nc.tensor.transpose(
                        tr[:, k * P:(k + 1) * P], mem16[:, mo, :], ident16
                    )
                nc.vector.tensor_copy(
                    memT[:, g * TRG * P:(g + 1) * TRG * P], tr[:]
                )
            for g in range(RT // TRG):
                tr = psum_tr.tile([P, TRG * P], f16, tag="tr")
                for k in range(TRG):
                    rt = g * TRG + k
                    nc.tensor.transpose(
                        tr[:, k * P:(k + 1) * P], q16[:, rt, :], ident16
                    )
                nc.vector.tensor_copy(
                    qT[:, g * TRG:(g + 1) * TRG, :].rearrange("p a b -> p (a b)"),
                    tr[:],
                )

            inv_all = persist.tile([P, RT], f32, tag="inv_all")

            # ---------------- main ----------------
            for blk in range(NB):
                expT = expTp.tile([P, MO, BLK * P], f16, tag="expT")
                for t in range(BLK):
                    rt = blk * BLK + t
                    e16 = expp.tile([P, M], f16, tag="e16")
                    sums = stats.tile([P, 2], f32, tag="sums")
                    # mm1 + exp in two PSUM halves (4 banks each)
                    for h in range(2):
                        sim = psum_sim.tile([P, M // 2], f32, tag="sim")
                        for j in range(4):
                            n0 = h * (M // 2) + j * 512
                            nc.tensor.matmul(
                                sim[:, j * 512:(j + 1) * 512],
                                lhsT=qT[:, rt, :],
                                rhs=memT[:, n0:n0 + 512],
                                start=True,
                                stop=True,
                            )
                        nc.scalar.activation(
                            e16[:, h * (M // 2):(h + 1) * (M // 2)],
                            sim[:],
                            EXP,
                            accum_out=sums[:, h:h + 1],
                        )
                    nc.vector.tensor_add(
                        sums[:, 0:1], sums[:, 0:1], sums[:, 1:2]
                    )
                    nc.vector.reciprocal(inv_all[:, rt:rt + 1], sums[:, 0:1])

                    # normalize -> f32 attention, DMA out
                    a32 = attnp.tile([P, M], f32, tag="a32")
                    if norm_on_act_every and rt % norm_on_act_every == 0:
                        nc.scalar.activation(
                            a32[:], e16[:], COPY, scale=inv_all[:, rt:rt + 1]
                        )
                    else:
                        nc.vector.tensor_scalar_mul(
                            a32[:], e16[:], inv_all[:, rt:rt + 1]
                        )
                    nc.sync.dma_start(attn[rt * P:(rt + 1) * P, :], a32[:])

                    # transpose e16 -> expT[:, :, t*128:(t+1)*128]
                    for g in range(MO // TRG):
                        tr = psum_tr.tile([P, TRG * P], f16, tag="tr")
                        for k in range(TRG):
                            mo = g * TRG + k
                            nc.tensor.transpose(
                                tr[:, k * P:(k + 1) * P],
                                e16[:, mo * P:(mo + 1) * P],
                                ident16,
                            )
                        nc.vector.tensor_copy(
                            expT[:, g * TRG:(g + 1) * TRG, t * P:(t + 1) * P],
                            tr[:].rearrange("p (a b) -> p a b", a=TRG),
                        )

                # mm2: mfT[d, rows=512] = sum_mo mem16[mo].T-contract @ expT[mo]
                pmf = psum_mf.tile([P, BLK * P], f32, tag="pmf")
                for mo in range(MO):
                    nc.tensor.matmul(
                        pmf[:],
                        lhsT=mem16[:, mo, :],
                        rhs=expT[:, mo, :],
                        start=(mo == 0),
                        stop=(mo == MO - 1),
                    )
                mfT = mfp.tile([P, BLK * P], f32, tag="mfT")
                nc.vector.tensor_copy(mfT[:], pmf[:])
                pmft = psum_mft.tile([P, BLK * P], f32, tag="pmft")
                for t in range(BLK):
                    nc.tensor.transpose(
                        pmft[:, t * P:(t + 1) * P],
                        mfT[:, t * P:(t + 1) * P],
                        ident32,
                    )
                mf_sb = mfp.tile([P, BLK, D], f32, tag="mf_sb")
                for t in range(BLK):
                    rt = blk * BLK + t
                    nc.vector.tensor_scalar_mul(
                        mf_sb[:, t, :],
                        pmft[:, t * P:(t + 1) * P],
                        inv_all[:, rt:rt + 1],
                    )
                nc.sync.dma_start(
                    mf.rearrange("(nb t p) d -> nb p t d", t=BLK, p=P)[blk],
                    mf_sb[:],
                )

    if fixup:
        _split_multiwaits(nc, mybir, max_waits=1)
    return nc


def _get_nc():
    if "nc" not in _CACHE:
        _CACHE["nc"] = build_nc()
    return _CACHE["nc"]


def kernel(query_features, memory):
    from concourse.bass_utils import run_bass_kernel_spmd

    nc = _get_nc()
    q = np.ascontiguousarray(query_features, dtype=np.float32).reshape(B * T, D)
    memn = np.ascontiguousarray(memory, dtype=np.float32)
    in_maps = [
        {"q": q[c * R:(c + 1) * R], "mem": memn} for c in range(N_CORES)
    ]
    res = run_bass_kernel_spmd(nc, in_maps, core_ids=list(range(N_CORES)))
    attn = np.concatenate(
        [res.results[c]["attn"] for c in range(N_CORES)], axis=0
    ).reshape(B, T, M)
    mf = np.concatenate(
        [res.results[c]["mf"] for c in range(N_CORES)], axis=0
    ).reshape(B, T, D)
    return mf, attn
